# revision 15
# baseline (speedup 1.0000x reference)
"""Gated-attention prefill layer on 8 Trainium2 cores (tensor-parallel over heads).

Model: Qwen3.5-style gated attention, H=16 q-heads / HKV=8 kv-heads, D=128,
partial RoPE (first 32 dims), q/k RMS-norm, sigmoid output gate, S=CTX=2048.

Sharding: core m owns q-heads {2m, 2m+1} and kv-head m. Each core computes its
QKV projection slice, attention for its 2 heads, and a partial o_proj
(Wo[:, m*256:(m+1)*256] @ attn_slice). The 8 fp-partials are summed on host.

Layout strategy on device (per core):
  - x kept as [C, S] (contraction on partitions) -> QKV matmul lhsT = x tile.
  - QKV out in [s, n] layout: n = [q0 q1 k g0 g1 v] (128 cols each).
  - RMS-norm + partial RoPE done in [s, d] layout (free-dim reductions).
  - q/k PE-transposed to [d, s] for the scores matmul; v stays [t, d] (= AV lhsT).
  - scores [s, t] in PSUM -> masked -> softmax on free dim -> probs bf16.
  - probs PE-transposed to [t, s] (2 s-tiles batched -> N=256 AV matmuls).
  - AV out [d, s] == o_proj rhs layout; gate transposed once per s-tile.
  - causal: scores/AV computed only for t-tiles <= s-pair (half the FLOPs).
All matmuls in bf16 (1 cycle/row on PE vs 4 for fp32).
"""

import sys

sys.path.insert(0, "/opt/trn_rl_repo")

import numpy as np
import ml_dtypes

import concourse.bass as bass
import concourse.tile as tile
import concourse.mybir as mybir
from concourse.vector_clock import ScopedClock

F32 = mybir.dt.float32
BF16 = mybir.dt.bfloat16
AF = mybir.ActivationFunctionType
AX = mybir.AxisListType

H, HKV, D, ROT = 16, 8, 128, 32
C, S, CTX, NL = 2048, 2048, 2048, 6
EPS = 1e-6
P = 128
NT = S // P          # 16 s-tiles
NP = NT // 2         # 8 s-pairs
INV_SQRT_D = float(D) ** -0.5


class _TC(tile.TileContext):
    """TileContext whose final drain splits its sem waits across nops.

    This container's walrus rejects >1 sync wait per instruction; stock Tile
    attaches the whole global clock to the tail drain.
    """

    def _drain_and_barrier(self, tick_clock, wait_clock):
        nops = [self.nc.sync.nop(nofuse=True) for _ in range(40)]
        drain_inst = self.nc.sync.drain()
        wait_clock.add_sem_waits(
            drain_inst.ins, ScopedClock({None: tick_clock.global_clock})
        )
        si = drain_inst.ins.sync_info
        waits = list(si.on_wait or [])
        if len(waits) > 1:
            extra, keep = waits[:-1], waits[-1:]
            try:
                si.on_wait.clear()
                for w in keep:
                    si.on_wait.append(w)
            except Exception:
                si.on_wait = keep
            assert len(extra) <= len(nops), f"{len(extra)} drain waits"
            for w, n in zip(extra, nops):
                nsi = n.ins.sync_info
                if nsi is None:
                    n.ins.sync_info = mybir.SyncInfo(on_wait=[w], on_update=[])
                elif nsi.on_wait is None:
                    nsi.on_wait = [w]
                else:
                    nsi.on_wait.append(w)
        self.nc.all_engine_barrier()
        assert self.sems is not None
        popped = self.nc._tile_sem_poison_stack.pop()
        assert popped is self._sem_poison
        self.nc.clear_and_free_semaphores(list(self.sems.allocated().values()))
        self.nc.all_engine_barrier()


class _Bass(bass.Bass):
    """Bass whose BIR serialization splits multi-wait instructions.

    This container's walrus accepts at most one sync wait per instruction;
    Tile's wait assignment can attach several. Extra waits are hoisted onto
    NoOps spliced immediately before the instruction on the same engine.
    """

    def to_json_bytes(self):
        import json as _json

        raw = super().to_json_bytes()
        d = _json.loads(raw)
        ctr = 0
        changed = False
        for f in d["functions"]:
            for bb in f["blocks"]:
                out = []
                for inst in bb["instructions"]:
                    si = inst.get("sync_info")
                    ow = (si or {}).get("on_wait") or []
                    if len(ow) > 1:
                        for w in ow[:-1]:
                            ctr += 1
                            out.append({
                                "debug": inst.get("debug", 0),
                                "engine": inst["engine"],
                                "ins": [], "outs": [],
                                "name": f"I-mwfix-{ctr}",
                                "opcode": "NoOp",
                                "sync_info": {"on_update": [],
                                              "on_wait": [w]},
                            })
                        si["on_wait"] = ow[-1:]
                        changed = True
                    out.append(inst)
                bb["instructions"] = out
        if not changed:
            return raw
        return _json.dumps(d).encode()


def build_program():
    nc = _Bass()

    xw = nc.declare_dram_parameter("xw", [C, S], BF16, isOutput=False)
    wqkv = nc.declare_dram_parameter("wqkv", [C, 768], BF16, isOutput=False)
    wo = nc.declare_dram_parameter("wo", [256, C], BF16, isOutput=False)
    cosr = nc.declare_dram_parameter("cosr", [P, NT * 48], F32, isOutput=False)
    sinr = nc.declare_dram_parameter("sinr", [P, NT * 48], F32, isOutput=False)
    w1 = nc.declare_dram_parameter("w1", [P, 384], F32, isOutput=False)
    mask2 = nc.declare_dram_parameter("mask2", [P, 512], F32, isOutput=False)
    eye = nc.declare_dram_parameter("eye", [P, P], BF16, isOutput=False)
    pout = nc.declare_dram_parameter("pout", [C, S], BF16, isOutput=True)
    kout = nc.declare_dram_parameter("kout", [S, D], F32, isOutput=True)
    vout = nc.declare_dram_parameter("vout", [S, D], F32, isOutput=True)

    with _TC(nc) as tc:
        # ---- persistent SBUF ----
        with tc.tile_pool(name="const", bufs=1) as constp, \
             tc.tile_pool(name="resid", bufs=1) as residp, \
             tc.tile_pool(name="tpsum", bufs=2, space="PSUM") as tpsum:
            cosres = constp.tile([P, NT * 48], F32, tag="cosres")
            nc.sync.dma_start(cosres[:], cosr[:])
            sinres = constp.tile([P, NT * 48], F32, tag="sinres")
            nc.sync.dma_start(sinres[:], sinr[:])
            w1res = constp.tile([P, 384], F32, tag="w1res")
            nc.sync.dma_start(w1res[:], w1[:])
            maskres = constp.tile([P, 512], F32, tag="maskres")
            nc.sync.dma_start(maskres[:], mask2[:])
            eyeres = constp.tile([P, P], BF16, tag="eyeres")
            nc.sync.dma_start(eyeres[:], eye[:])
            epst = constp.tile([P, 1], F32, tag="epst")
            nc.vector.memset(epst[:], EPS)
            # Wo^T resident: [:, 0:2048] = local head 0 rows, [:, 2048:] = head 1
            wores = constp.tile([P, 2 * C], BF16, tag="wores")
            nc.sync.dma_start(wores[:, 0:C], wo[0:P, :])
            nc.sync.dma_start(wores[:, C:2 * C], wo[P:256, :])

            qds = residp.tile([P, 2 * S], BF16, tag="qds")    # q [d, s] per head
            kdt = residp.tile([P, S], BF16, tag="kdt")        # k [d, t]
            vres = residp.tile([P, S], BF16, tag="vres")      # v [t, d] tiles
            gres = residp.tile([P, NT * 256], F32, tag="gres")  # gate [s, d] (h0,h1 per s-tile)
            att = residp.tile([P, 2 * S], BF16, tag="att")    # attn [d, s] per head

            # ================= phase 1: QKV + norm + rope =================
            with tc.tile_pool(name="xres", bufs=1) as xpool, \
                 tc.tile_pool(name="wres", bufs=1) as wpool, \
                 tc.tile_pool(name="qkvps", bufs=2, space="PSUM") as qkvps, \
                 tc.tile_pool(name="work1", bufs=3) as work1:
                xres = xpool.tile([P, NT * S], BF16, tag="xres")
                for ct in range(NT):
                    nc.sync.dma_start(
                        xres[:, ct * S:(ct + 1) * S], xw[ct * P:(ct + 1) * P, :]
                    )
                wres = wpool.tile([P, NT * 768], BF16, tag="wres")
                for ct in range(NT):
                    nc.sync.dma_start(
                        wres[:, ct * 768:(ct + 1) * 768],
                        wqkv[ct * P:(ct + 1) * P, :],
                    )

                for st in range(NT):
                    ps = qkvps.tile([P, 768], F32, tag="qkvpsum")
                    for half, (n0, n1) in enumerate(((0, 512), (512, 768))):
                        for ct in range(NT):
                            lhs = xres[:, ct * S + st * P: ct * S + (st + 1) * P]
                            nc.tensor.matmul(
                                ps[:, n0:n1], lhs,
                                wres[:, ct * 768 + n0: ct * 768 + n1],
                                start=(ct == 0), stop=(ct == NT - 1),
                            )

                    # sum of squares over d for q0,q1,k
                    sq = work1.tile([P, 384], F32, tag="sq")
                    nc.scalar.square(sq[:], ps[:, 0:384])
                    ssq = work1.tile([P, 3], F32, tag="ssq")
                    nc.vector.reduce_sum(
                        ssq[:].rearrange("p (h x) -> p h x", x=1),
                        sq[:].rearrange("p (h d) -> p h d", d=P),
                        axis=AX.X,
                    )
                    rms = work1.tile([P, 3], F32, tag="rms")
                    nc.scalar.activation(rms[:], ssq[:], AF.Sqrt,
                                         bias=epst[:], scale=1.0 / D)
                    rinv = work1.tile([P, 3], F32, tag="rinv")
                    nc.vector.reciprocal(rinv[:], rms[:])
                    # fold 1/sqrt(D) into q scales
                    nc.vector.tensor_scalar_mul(rinv[:, 0:2], rinv[:, 0:2],
                                                INV_SQRT_D)

                    # normed = psum * (1+w) * rinv  (fp32, [s, d])
                    A = work1.tile([P, 384], F32, tag="A")
                    nc.vector.tensor_mul(A[:], ps[:, 0:384], w1res[:])
                    for j in range(3):
                        nc.vector.tensor_scalar_mul(
                            A[:, j * P:(j + 1) * P], A[:, j * P:(j + 1) * P],
                            rinv[:, j:j + 1],
                        )

                    # partial rope on first 32 dims (q0,q1,k batched via 3D AP)
                    A3 = A[:].rearrange("p (h d) -> p h d", d=P)
                    U = work1.tile([P, 192], F32, tag="U")
                    U3 = U[:].rearrange("p (h e) -> p h e", e=64)
                    c3 = cosres[:, st * 48:(st + 1) * 48].rearrange(
                        "p (h e) -> p h e", e=16)
                    s3 = sinres[:, st * 48:(st + 1) * 48].rearrange(
                        "p (h e) -> p h e", e=16)
                    nc.vector.tensor_mul(U3[:, :, 0:16], A3[:, :, 16:32], s3)
                    nc.vector.tensor_mul(U3[:, :, 16:32], A3[:, :, 16:32], c3)
                    nc.vector.tensor_mul(U3[:, :, 32:48], A3[:, :, 0:16], c3)
                    nc.vector.tensor_mul(U3[:, :, 48:64], A3[:, :, 0:16], s3)
                    nc.vector.tensor_sub(A3[:, :, 0:16], U3[:, :, 32:48],
                                         U3[:, :, 0:16])
                    nc.vector.tensor_add(A3[:, :, 16:32], U3[:, :, 16:32],
                                         U3[:, :, 48:64])

                    # k cache out (fp32, post-rope)
                    nc.sync.dma_start(kout[st * P:(st + 1) * P, :],
                                      A[:, 256:384])
                    # v: fp32 for cache, bf16 for AV
                    vf = work1.tile([P, P], F32, tag="vf")
                    nc.vector.tensor_copy(vf[:], ps[:, 640:768])
                    nc.sync.dma_start(vout[st * P:(st + 1) * P, :], vf[:])
                    nc.vector.tensor_copy(vres[:, st * P:(st + 1) * P],
                                          ps[:, 640:768])
                    # gates stay fp32 [s, d], both heads adjacent per s-tile
                    nc.vector.tensor_copy(gres[:, st * 256:(st + 1) * 256],
                                          ps[:, 384:640])

                    # bf16 cast + PE transpose q0,q1,k -> [d, s]
                    qkbf = work1.tile([P, 384], BF16, tag="qkbf")
                    nc.vector.tensor_copy(qkbf[:], A[:])
                    for j, dst in ((0, qds[:, st * P:(st + 1) * P]),
                                   (1, qds[:, S + st * P: S + (st + 1) * P]),
                                   (2, kdt[:, st * P:(st + 1) * P])):
                        tp = tpsum.tile([P, P], BF16, tag="tp")
                        nc.tensor.transpose(tp[:], qkbf[:, j * P:(j + 1) * P],
                                            eyeres[:])
                        nc.vector.tensor_copy(dst, tp[:])

            # ================= phase 2: attention =================
            with tc.tile_pool(name="scps", bufs=2, space="PSUM") as scps, \
                 tc.tile_pool(name="avps", bufs=2, space="PSUM") as avps, \
                 tc.tile_pool(name="work2", bufs=3) as work2, \
                 tc.tile_pool(name="sbsc", bufs=2) as sbsc, \
                 tc.tile_pool(name="ptp", bufs=2) as ptp, \
                 tc.tile_pool(name="pbpool", bufs=4) as pbpool:
                for h in range(2):
                    for pi in range(NP):
                        T = (2 * pi + 2) * P
                        probs = []
                        for sl in range(2):
                            st = 2 * pi + sl
                            sb = sbsc.tile([P, S], F32, tag="scores")
                            nch = (T + 511) // 512
                            for ci in range(nch):
                                c0 = ci * 512
                                cw = min(512, T - c0)
                                sps = scps.tile([P, 512], F32, tag="scps")
                                nc.tensor.matmul(
                                    sps[:, 0:cw],
                                    qds[:, h * S + st * P: h * S + (st + 1) * P],
                                    kdt[:, c0:c0 + cw],
                                    start=True, stop=True,
                                )
                                m0 = T - 256  # mask region start
                                lo, hi = c0, c0 + cw
                                pe = min(max(lo, m0), hi)
                                if pe > lo:
                                    nc.vector.tensor_copy(sb[:, lo:pe],
                                                          sps[:, 0:pe - lo])
                                if hi > pe:
                                    nc.vector.tensor_add(
                                        sb[:, pe:hi],
                                        sps[:, pe - lo:hi - lo],
                                        maskres[:, sl * 256 + (pe - m0):
                                                sl * 256 + (hi - m0)],
                                    )
                            # softmax over [0, T)
                            m = work2.tile([P, 1], F32, tag="max")
                            nc.vector.reduce_max(m[:], sb[:, 0:T], axis=AX.X)
                            nm = work2.tile([P, 1], F32, tag="negmax")
                            nc.vector.tensor_scalar_mul(nm[:], m[:], -1.0)
                            pb = pbpool.tile([P, S], BF16, tag="probs")
                            lsum = work2.tile([P, 1], F32, tag="lsum")
                            nc.scalar.activation(pb[:, 0:T], sb[:, 0:T], AF.Exp,
                                                 bias=nm[:], scale=1.0,
                                                 accum_out=lsum[:])
                            linv = work2.tile([P, 1], F32, tag="linv")
                            nc.vector.reciprocal(linv[:], lsum[:])
                            probs.append(pb)

                            # gate: sigmoid * (1/l), then transpose to [d, s]
                            gs = work2.tile([P, P], F32, tag="gs")
                            nc.scalar.activation(
                                gs[:],
                                gres[:, st * 256 + h * P: st * 256 + (h + 1) * P],
                                AF.Sigmoid)
                            gsc = work2.tile([P, P], BF16, tag="gsc")
                            nc.vector.tensor_scalar_mul(gsc[:], gs[:], linv[:])
                            tpg = tpsum.tile([P, P], BF16, tag="tp")
                            nc.tensor.transpose(tpg[:], gsc[:], eyeres[:])
                            gT = work2.tile([P, P], BF16, tag="gT")
                            nc.vector.tensor_copy(gT[:], tpg[:])
                            probs.append(gT)

                        # transpose probs to [t, s] (both s-tiles batched),
                        # then run the AV accumulation group uninterrupted
                        ntt = 2 * pi + 2
                        ptall = ptp.tile([P, NT * 256], BF16, tag="ptall")
                        for tt in range(ntt):
                            for sl in range(2):
                                tp2 = tpsum.tile([P, P], BF16, tag="tp")
                                nc.tensor.transpose(
                                    tp2[:],
                                    probs[2 * sl][:, tt * P:(tt + 1) * P],
                                    eyeres[:])
                                nc.vector.tensor_copy(
                                    ptall[:, tt * 256 + sl * P:
                                          tt * 256 + (sl + 1) * P], tp2[:])
                        av = avps.tile([P, 256], F32, tag="avps")
                        for tt in range(ntt):
                            nc.tensor.matmul(av[:], vres[:, tt * P:(tt + 1) * P],
                                             ptall[:, tt * 256:(tt + 1) * 256],
                                             start=(tt == 0),
                                             stop=(tt == ntt - 1))
                        # gate-mult -> att [d, s]
                        for sl in range(2):
                            st = 2 * pi + sl
                            nc.vector.tensor_mul(
                                att[:, h * S + st * P: h * S + (st + 1) * P],
                                av[:, sl * P:(sl + 1) * P],
                                probs[2 * sl + 1][:])

            # ================= phase 3: o_proj partial =================
            with tc.tile_pool(name="ops", bufs=2, space="PSUM") as opsp, \
                 tc.tile_pool(name="outp", bufs=3) as outp:
                for ct in range(NT):
                    for sc in range(4):
                        op = opsp.tile([P, 512], F32, tag="opsum")
                        for h in range(2):
                            nc.tensor.matmul(
                                op[:],
                                wores[:, h * C + ct * P: h * C + (ct + 1) * P],
                                att[:, h * S + sc * 512: h * S + (sc + 1) * 512],
                                start=(h == 0), stop=(h == 1),
                            )
                        ob = outp.tile([P, 512], BF16, tag="ob")
                        nc.vector.tensor_copy(ob[:], op[:])
                        nc.sync.dma_start(
                            pout[ct * P:(ct + 1) * P, sc * 512:(sc + 1) * 512],
                            ob[:])
    return nc


def make_core_inputs(hidden_states, kv_cache, causal_mask, position_ids,
                     attn_layer_idx, cos_cached, sin_cached, Wq, Wk, Wv, Wo,
                     q_norm_w, k_norm_w):
    """Build the 8 per-core input dicts (host-side sharding + layout prep)."""
    bf = ml_dtypes.bfloat16
    x = np.asarray(hidden_states, np.float32)[0, :, 0, :]          # [C, S]
    x_bf = x.astype(bf)
    pos = np.asarray(position_ids).astype(np.int64)
    cos_pos = np.asarray(cos_cached, np.float32)[pos]              # [S, 32]
    sin_pos = np.asarray(sin_cached, np.float32)[pos]
    c16 = cos_pos[:, :16].reshape(NT, P, 16).transpose(1, 0, 2)    # [P, NT, 16]
    s16 = sin_pos[:, :16].reshape(NT, P, 16).transpose(1, 0, 2)
    cosr = np.repeat(c16[:, :, None, :], 3, axis=2).reshape(P, NT * 48)
    sinr = np.repeat(s16[:, :, None, :], 3, axis=2).reshape(P, NT * 48)
    cosr = np.ascontiguousarray(cosr, np.float32)
    sinr = np.ascontiguousarray(sinr, np.float32)

    qw = np.asarray(q_norm_w, np.float32)
    kw = np.asarray(k_norm_w, np.float32)
    w1 = np.broadcast_to(
        np.concatenate([1.0 + qw, 1.0 + qw, 1.0 + kw])[None, :], (P, 384))
    w1 = np.ascontiguousarray(w1, np.float32)

    tl = np.where(np.tril(np.ones((P, P), bool)), 0.0, -1e9).astype(np.float32)
    m_even = np.concatenate([tl, np.full((P, P), -1e9, np.float32)], axis=1)
    m_odd = np.concatenate([np.zeros((P, P), np.float32), tl], axis=1)
    mask2 = np.ascontiguousarray(np.concatenate([m_even, m_odd], axis=1))

    eye = np.eye(P, dtype=bf)

    Wq = np.asarray(Wq, np.float32)
    Wk = np.asarray(Wk, np.float32)
    Wv = np.asarray(Wv, np.float32)
    Wo = np.asarray(Wo, np.float32)

    in_maps = []
    for m in range(8):
        h0, h1 = 2 * m, 2 * m + 1
        wc = np.concatenate([
            Wq[h0 * 256: h0 * 256 + 128],
            Wq[h1 * 256: h1 * 256 + 128],
            Wk[m * 128: (m + 1) * 128],
            Wq[h0 * 256 + 128: h0 * 256 + 256],
            Wq[h1 * 256 + 128: h1 * 256 + 256],
            Wv[m * 128: (m + 1) * 128],
        ], axis=0)                                                  # [768, C]
        wqkv = np.ascontiguousarray(wc.T).astype(bf)                # [C, 768]
        wom = np.ascontiguousarray(Wo[:, m * 256:(m + 1) * 256].T).astype(bf)
        in_maps.append({
            "xw": x_bf, "wqkv": wqkv, "wo": wom,
            "cosr": cosr, "sinr": sinr, "w1": w1, "mask2": mask2, "eye": eye,
        })
    return in_maps


_NC_CACHE = {}


def kernel(hidden_states, kv_cache, causal_mask, position_ids, attn_layer_idx,
           cos_cached, sin_cached, Wq, Wk, Wv, Wo, q_norm_w, k_norm_w):
    from concourse.bass_utils import run_bass_kernel_spmd

    if "nc" not in _NC_CACHE:
        _NC_CACHE["nc"] = build_program()
    nc = _NC_CACHE["nc"]

    in_maps = make_core_inputs(hidden_states, kv_cache, causal_mask,
                               position_ids, attn_layer_idx, cos_cached,
                               sin_cached, Wq, Wk, Wv, Wo, q_norm_w, k_norm_w)
    import time as _time
    _t0 = _time.perf_counter()
    res = run_bass_kernel_spmd(nc, in_maps, list(range(8)))
    _NC_CACHE["spmd_s"] = _time.perf_counter() - _t0
    _NC_CACHE["last_results"] = res

    out = np.zeros((C, S), np.float32)
    for m in range(8):
        out += res.results[m]["pout"].astype(np.float32)
    output = out.reshape(1, C, 1, S)

    cache = np.array(np.asarray(kv_cache, np.float32), copy=True)
    li = int(np.asarray(attn_layer_idx)) * 2
    p0 = int(np.asarray(position_ids)[0])
    for m in range(8):
        cache[li, m, p0:p0 + S, :] = res.results[m]["kout"]
        cache[li + 1, m, p0:p0 + S, :] = res.results[m]["vout"]
    return output, cache
